# revision 4
# baseline (speedup 1.0000x reference)
"""Trainium2 Bass kernel for nn_CapsuleSubLayer (capsule routing).

Math (per head h):
  uh[b,d,j] = sum_s W[h,d,j,s] * x[h,b,s,d]            (batched matmul over d)
  3 routing iterations of softmax / weighted-sum / squash / logit update
  out[b,d,n,h] = v[h,b,d]  (broadcast over n)

Sharding: heads are fully independent -> 2 heads per NeuronCore on 8 cores.

PE packing: two d's per matmul. lhsT = [x_d0 | x_d1] ([128s, 128], which
triggers the fast 128-column weight load), rhs = [w_d0 | w_d1] ([128s, 32]);
the psum out [128, 32] holds the two diagonal blocks uh_d0 = out[0:64, 0:16]
and uh_d1 = out[64:128, 16:32] (off-diagonal blocks are discarded). This
halves the LDWEIGHTS count vs one matmul per d and doubles weight-load rate.

Routing runs on all 128 partitions (partition = (d parity, b)), halving
vector-engine time vs a 64-partition layout: reductions over n are inner-free
reduces; the mean over b uses a block-diagonal ones matmul on the PE which
also leaves the result replicated across each 64-partition half (exactly the
layout the next softmax needs).

Host-side pre-permute (fp16; PSUM accumulation stays fp32):
  xt[h, p, c, d, b] = x[h, b, c*128+p, d]
  wt[h, p, c, d, n] = W[h, d, n, c*128+p]
so each DMA descriptor is a contiguous DG*64*2 (x) / DG*16*2 (w) byte run.
"""

import os
import sys

import numpy as np

for _p in ("/opt/trn_rl_repo",):
    if _p not in sys.path:
        sys.path.insert(0, _p)

from contextlib import ExitStack

import concourse.bass as bass
import concourse.tile as tile
from concourse import bacc, mybir
from concourse.bass_utils import run_bass_kernel_spmd

F32 = mybir.dt.float32

H, B, S, D, N = 16, 64, 1024, 64, 16
NCORES = 8
H_LOC = H // NCORES  # 2 heads per core
C = S // 128  # 8 contraction chunks
P = D // 2  # d-pairs per head

IN_DT = mybir.dt.float16
IN_NP = np.float16

_cache = {}


def _build(num_routing: int, repeat: int = 1, DG: int = 32):
    nc = bacc.Bacc(
        "TRN2", target_bir_lowering=False, debug=False, num_devices=NCORES
    )
    xt = nc.dram_tensor("xt", [H_LOC, 128, C, D * B], IN_DT, kind="ExternalInput").ap()
    wt = nc.dram_tensor("wt", [H_LOC, 128, C, D * N], IN_DT, kind="ExternalInput").ap()
    ones = nc.dram_tensor("ones", [128, 128], mybir.dt.float16, kind="ExternalInput").ap()
    vout = nc.dram_tensor("vout", [128, H_LOC * P], F32, kind="ExternalOutput").ap()

    PG = DG // 2  # pairs per DMA batch

    with ExitStack() as ctx:
        tc = ctx.enter_context(tile.TileContext(nc))
        xpool = ctx.enter_context(tc.tile_pool(name="xp", bufs=3))
        wpool = ctx.enter_context(tc.tile_pool(name="wp", bufs=2))
        pspool = ctx.enter_context(tc.tile_pool(name="ps", bufs=3, space="PSUM"))
        bppool = ctx.enter_context(tc.tile_pool(name="bp", bufs=2, space="PSUM"))
        uhpool = ctx.enter_context(tc.tile_pool(name="uh", bufs=3))
        rpool = ctx.enter_context(tc.tile_pool(name="rt", bufs=3))
        spool = ctx.enter_context(tc.tile_pool(name="sm", bufs=6))
        singles = ctx.enter_context(tc.tile_pool(name="sg", bufs=1))

        ones_sb = singles.tile([128, 128], mybir.dt.float16)
        nc.sync.dma_start(out=ones_sb, in_=ones)

        def routing(uh, vout_slice, RC):
            """3-iteration dynamic routing on a [128, RC, N] uh chunk
            (partition = (d parity, b)). b_logits live in PSUM (bl_ps),
            accumulated by block-diagonal ones*N/B matmuls."""
            bl_ps = bppool.tile([128, RC, N], F32, tag="bl")
            for it in range(num_routing):
                if it == 0:
                    s_raw = spool.tile([128, RC, 1], F32, tag="sr")
                    nc.vector.reduce_sum(s_raw, uh, mybir.AxisListType.X)
                    scale = 1.0 / N
                else:
                    e = rpool.tile([128, RC, N], F32, tag="e")
                    nc.scalar.activation(e, bl_ps, mybir.ActivationFunctionType.Exp)
                    esum = spool.tile([128, RC, 1], F32, tag="es")
                    nc.vector.reduce_sum(esum, e, mybir.AxisListType.X)
                    erec = spool.tile([128, RC, 1], F32, tag="er")
                    nc.vector.reciprocal(erec, esum)
                    cu = rpool.tile([128, RC, N], F32, tag="cu")
                    nc.vector.tensor_mul(cu, e, uh)
                    s_raw = spool.tile([128, RC, 1], F32, tag="sr")
                    csum = spool.tile([128, RC, 1], F32, tag="cs")
                    nc.vector.reduce_sum(csum, cu, mybir.AxisListType.X)
                    nc.vector.tensor_mul(s_raw, csum, erec)
                    scale = 1.0

                # squash: v = s*|s| / (1 + s^2), s = s_raw*scale
                # critical path: Square -> +1 -> recip -> t1 -> v
                # (Abs and s_sc hang off s_raw in parallel)
                m = spool.tile([128, RC, 1], F32, tag="m")
                nc.scalar.activation(
                    m, s_raw, mybir.ActivationFunctionType.Abs, scale=scale
                )
                msq = spool.tile([128, RC, 1], F32, tag="mq")
                nc.scalar.activation(
                    msq, s_raw, mybir.ActivationFunctionType.Square, scale=scale
                )
                if scale != 1.0:
                    s_sc = spool.tile([128, RC, 1], F32, tag="ssc")
                    nc.scalar.mul(s_sc, s_raw, scale)
                else:
                    s_sc = s_raw
                den = spool.tile([128, RC, 1], F32, tag="dn")
                nc.vector.tensor_scalar_add(den, msq, 1.0)
                rec = spool.tile([128, RC, 1], F32, tag="rc")
                nc.vector.reciprocal(rec, den)
                t1 = spool.tile([128, RC, 1], F32, tag="t1")
                nc.vector.tensor_mul(t1, m, rec)
                v = spool.tile([128, RC, 1], F32, tag="v")
                nc.vector.tensor_mul(v, t1, s_sc)

                if it < num_routing - 1:
                    uv = rpool.tile([128, RC, N], mybir.dt.float16, tag="uv")
                    nc.vector.tensor_mul(uv, uh, v.to_broadcast((128, RC, N)))
                    # ones_sb holds block-diag N/B, so this accumulates
                    # bl += (N/B) * sum_b uh*v per d-parity half in PSUM
                    nc.tensor.matmul(
                        bl_ps,
                        ones_sb,
                        uv,
                        start=(it == 0),
                        stop=(it == num_routing - 2),
                    )
                else:
                    vo = spool.tile([128, RC], F32, tag="vo")
                    nc.vector.tensor_copy(out=vo, in_=v[:, :, 0])
                    nc.sync.dma_start(out=vout_slice, in_=vo)

        # Routing-chunk schedule in d-pair units: smaller chunks late so the
        # final routing chain (which trails the last DMA) is short.
        sched = {0: [(0, 16), (16, 16)], 1: [(0, 16), (16, 8), (24, 8)]}

        for rep in range(repeat):
          for h in range(H_LOC):
            ps = None
            uh = None
            chunk = dict()
            for q0, qsz in sched[h]:
                for q in range(q0, q0 + qsz):
                    chunk[q] = (q0, qsz)
            w_t = wpool.tile([128, C, D * N], IN_DT)
            nc.sync.dma_start(out=w_t, in_=wt[h])
            for dg in range(D // DG):
                x_t = xpool.tile([128, C, DG * B], IN_DT)
                nc.sync.dma_start(
                    out=x_t,
                    in_=xt[h, :, :, dg * DG * B : (dg + 1) * DG * B],
                )
                for ql in range(PG):
                    q = dg * PG + ql
                    q0, qsz = chunk[q]
                    if q == q0:
                        ps = pspool.tile([128, qsz, 2 * N], F32, tag="ps")
                        uh = uhpool.tile([128, qsz, N], F32, tag="uh")
                    for c in range(C):
                        nc.tensor.matmul(
                            ps[:, q - q0, :],
                            x_t[:, c, 2 * ql * B : (2 * ql + 2) * B],
                            w_t[:, c, (dg * DG + 2 * ql) * N : (dg * DG + 2 * ql + 2) * N],
                            start=(c == 0),
                            stop=(c == C - 1),
                        )
                    if q == q0 + qsz - 1:
                        # extract diagonal blocks: uh[p<64] = ps[.., 0:N],
                        # uh[p>=64] = ps[.., N:2N]
                        nc.vector.tensor_copy(
                            out=uh[0:64], in_=ps[0:64, :, 0:N]
                        )
                        nc.vector.tensor_copy(
                            out=uh[64:128], in_=ps[64:128, :, N : 2 * N]
                        )
                        r0 = h * P + q0
                        routing(uh, vout[:, r0 : r0 + qsz], qsz)
    nc.finalize()
    return nc


def _prep_core(x, W, k):
    xs = x[2 * k : 2 * k + 2]  # [2, B, S, D]
    # xt[h, p, c, d, b] = x[h, b, c*128+p, d]
    xt = np.ascontiguousarray(
        xs.reshape(H_LOC, B, C, 128, D).transpose(0, 3, 2, 4, 1).astype(IN_NP)
    ).reshape(H_LOC, 128, C, D * B)
    ws = W[2 * k : 2 * k + 2]  # [2, D, N, S]
    # wt[h, p, c, d, n] = W[h, d, n, c*128+p]
    wt = np.ascontiguousarray(
        ws.reshape(H_LOC, D, N, C, 128).transpose(0, 4, 3, 1, 2).astype(IN_NP)
    ).reshape(H_LOC, 128, C, D * N)
    return xt, wt


def _make_ones():
    o = np.zeros((128, 128), dtype=np.float16)
    o[:64, :64] = np.float16(float(N) / B)
    o[64:, 64:] = np.float16(float(N) / B)
    return o


def unshard(vouts):
    """vouts: list of 8 arrays [128, H_LOC*P] -> full [B, D, N, H] output."""
    v_full = np.empty((H, B, D), dtype=np.float32)
    for k in range(NCORES):
        r = vouts[k]  # [128, H_LOC*P]; row = (d%2)*64 + b, col = h*P + d//2
        rr = np.asarray(r).reshape(2, B, H_LOC, P)  # [par, b, h, q]
        for h in range(H_LOC):
            v = np.empty((B, D), dtype=np.float32)
            v[:, 0::2] = rr[0, :, h, :]
            v[:, 1::2] = rr[1, :, h, :]
            v_full[2 * k + h] = v
    out = np.broadcast_to(
        v_full.transpose(1, 2, 0)[:, :, None, :], (B, D, N, H)
    )
    return np.ascontiguousarray(out)


def kernel(x, W, num_routing):
    x = np.asarray(x, dtype=np.float32)
    W = np.asarray(W, dtype=np.float32)
    nr = int(num_routing)
    if nr not in _cache:
        _cache[nr] = _build(nr)
    nc = _cache[nr]

    ones = _make_ones()
    in_maps = []
    for k in range(NCORES):
        xt, wt = _prep_core(x, W, k)
        in_maps.append({"xt": xt, "wt": wt, "ones": ones})

    kernel.last_in_maps = in_maps
    res = run_bass_kernel_spmd(
        nc,
        in_maps,
        core_ids=list(range(NCORES)),
        trace=bool(int(os.environ.get("KERNEL_TRACE", "0"))),
    )
    kernel.last_result = res

    return unshard([res.results[k]["vout"] for k in range(NCORES)])


# revision 5
# speedup vs baseline: 1.0074x; 1.0074x over previous
"""Trainium2 Bass kernel for nn_CapsuleSubLayer (capsule routing).

Math (per head h):
  uh[b,d,j] = sum_s W[h,d,j,s] * x[h,b,s,d]            (batched matmul over d)
  3 routing iterations of softmax / weighted-sum / squash / logit update
  out[b,d,n,h] = v[h,b,d]  (broadcast over n)

Sharding: heads are fully independent -> 2 heads per NeuronCore on 8 cores.

PE packing: two d's per matmul. lhsT = [x_d0 | x_d1] ([128s, 128], which
triggers the fast 128-column weight load), rhs = [w_d0 | w_d1] ([128s, 32]);
the psum out [128, 32] holds the two diagonal blocks uh_d0 = out[0:64, 0:16]
and uh_d1 = out[64:128, 16:32] (off-diagonal blocks are discarded). This
halves the LDWEIGHTS count vs one matmul per d and doubles weight-load rate.

Routing runs on all 128 partitions (partition = (d parity, b)), halving
vector-engine time vs a 64-partition layout: reductions over n are inner-free
reduces; the mean over b uses a block-diagonal ones matmul on the PE which
also leaves the result replicated across each 64-partition half (exactly the
layout the next softmax needs).

Host-side pre-permute (fp16; PSUM accumulation stays fp32):
  xt[h, p, c, d, b] = x[h, b, c*128+p, d]
  wt[h, p, c, d, n] = W[h, d, n, c*128+p]
so each DMA descriptor is a contiguous DG*64*2 (x) / DG*16*2 (w) byte run.
"""

import os
import sys

import numpy as np

for _p in ("/opt/trn_rl_repo",):
    if _p not in sys.path:
        sys.path.insert(0, _p)

from contextlib import ExitStack

import concourse.bass as bass
import concourse.tile as tile
from concourse import bacc, mybir
from concourse.bass_utils import run_bass_kernel_spmd

F32 = mybir.dt.float32

H, B, S, D, N = 16, 64, 1024, 64, 16
NCORES = 8
H_LOC = H // NCORES  # 2 heads per core
C = S // 128  # 8 contraction chunks
P = D // 2  # d-pairs per head

IN_DT = mybir.dt.float16
IN_NP = np.float16

_cache = {}


def _build(num_routing: int, repeat: int = 1, DG: int = 32):
    nc = bacc.Bacc(
        "TRN2", target_bir_lowering=False, debug=False, num_devices=NCORES
    )
    xt = nc.dram_tensor("xt", [H_LOC, 128, C, D * B], IN_DT, kind="ExternalInput").ap()
    wt = nc.dram_tensor("wt", [H_LOC, 128, C, D * N], IN_DT, kind="ExternalInput").ap()
    ones = nc.dram_tensor("ones", [128, 128], mybir.dt.float16, kind="ExternalInput").ap()
    vout = nc.dram_tensor("vout", [128, H_LOC * P], F32, kind="ExternalOutput").ap()

    PG = DG // 2  # pairs per DMA batch

    with ExitStack() as ctx:
        tc = ctx.enter_context(tile.TileContext(nc))
        xpool = ctx.enter_context(tc.tile_pool(name="xp", bufs=3))
        wpool = ctx.enter_context(tc.tile_pool(name="wp", bufs=2))
        pspool = ctx.enter_context(tc.tile_pool(name="ps", bufs=3, space="PSUM"))
        bppool = ctx.enter_context(tc.tile_pool(name="bp", bufs=2, space="PSUM"))
        uhpool = ctx.enter_context(tc.tile_pool(name="uh", bufs=3))
        rpool = ctx.enter_context(tc.tile_pool(name="rt", bufs=3))
        spool = ctx.enter_context(tc.tile_pool(name="sm", bufs=6))
        singles = ctx.enter_context(tc.tile_pool(name="sg", bufs=1))
        vopool = ctx.enter_context(tc.tile_pool(name="vo", bufs=2))

        ones_sb = singles.tile([128, 128], mybir.dt.float16)
        nc.sync.dma_start(out=ones_sb, in_=ones)

        def routing(uh, vout_slice, RC):
            """3-iteration dynamic routing on a [128, RC, N] uh chunk
            (partition = (d parity, b)). b_logits live in PSUM (bl_ps),
            accumulated by block-diagonal ones*N/B matmuls."""
            bl_ps = bppool.tile([128, RC, N], F32, tag="bl")
            for it in range(num_routing):
                if it == 0:
                    s_raw = spool.tile([128, RC, 1], F32, tag="sr")
                    nc.vector.reduce_sum(s_raw, uh, mybir.AxisListType.X)
                    scale = 1.0 / N
                else:
                    e = rpool.tile([128, RC, N], F32, tag="e")
                    nc.scalar.activation(e, bl_ps, mybir.ActivationFunctionType.Exp)
                    esum = spool.tile([128, RC, 1], F32, tag="es")
                    nc.vector.reduce_sum(esum, e, mybir.AxisListType.X)
                    erec = spool.tile([128, RC, 1], F32, tag="er")
                    nc.vector.reciprocal(erec, esum)
                    cu = rpool.tile([128, RC, N], F32, tag="cu")
                    nc.vector.tensor_mul(cu, e, uh)
                    s_raw = spool.tile([128, RC, 1], F32, tag="sr")
                    csum = spool.tile([128, RC, 1], F32, tag="cs")
                    nc.vector.reduce_sum(csum, cu, mybir.AxisListType.X)
                    nc.vector.tensor_mul(s_raw, csum, erec)
                    scale = 1.0

                # squash: v = s*|s| / (1 + s^2), s = s_raw*scale
                # critical path: Square -> +1 -> recip -> t1 -> v
                # (Abs and s_sc hang off s_raw in parallel)
                m = spool.tile([128, RC, 1], F32, tag="m")
                nc.scalar.activation(
                    m, s_raw, mybir.ActivationFunctionType.Abs, scale=scale
                )
                msq = spool.tile([128, RC, 1], F32, tag="mq")
                nc.scalar.activation(
                    msq, s_raw, mybir.ActivationFunctionType.Square, scale=scale
                )
                if scale != 1.0:
                    s_sc = spool.tile([128, RC, 1], F32, tag="ssc")
                    nc.scalar.mul(s_sc, s_raw, scale)
                else:
                    s_sc = s_raw
                den = spool.tile([128, RC, 1], F32, tag="dn")
                nc.vector.tensor_scalar_add(den, msq, 1.0)
                rec = spool.tile([128, RC, 1], F32, tag="rc")
                nc.vector.reciprocal(rec, den)
                t1 = spool.tile([128, RC, 1], F32, tag="t1")
                nc.vector.tensor_mul(t1, m, rec)
                v = spool.tile([128, RC, 1], F32, tag="v")
                nc.vector.tensor_mul(v, t1, s_sc)

                if it < num_routing - 1:
                    uv = rpool.tile([128, RC, N], mybir.dt.float16, tag="uv")
                    nc.vector.tensor_mul(uv, uh, v.to_broadcast((128, RC, N)))
                    # ones_sb holds block-diag N/B, so this accumulates
                    # bl += (N/B) * sum_b uh*v per d-parity half in PSUM
                    nc.tensor.matmul(
                        bl_ps,
                        ones_sb,
                        uv,
                        start=(it == 0),
                        stop=(it == num_routing - 2),
                    )
                else:
                    nc.vector.tensor_copy(out=vout_slice, in_=v[:, :, 0])

        # Routing-chunk schedule in d-pair units: smaller chunks late so the
        # final routing chain (which trails the last DMA) is short.
        sched = {0: [(0, 16), (16, 16)], 1: [(0, 16), (16, 8), (24, 8)]}

        for rep in range(repeat):
          for h in range(H_LOC):
            ps = None
            uh = None
            chunk = dict()
            for q0, qsz in sched[h]:
                for q in range(q0, q0 + qsz):
                    chunk[q] = (q0, qsz)
            vo_full = vopool.tile([128, P], F32, tag="vf")
            w_t = wpool.tile([128, C, D * N], IN_DT)
            nc.sync.dma_start(out=w_t, in_=wt[h])
            for dg in range(D // DG):
                x_t = xpool.tile([128, C, DG * B], IN_DT)
                nc.sync.dma_start(
                    out=x_t,
                    in_=xt[h, :, :, dg * DG * B : (dg + 1) * DG * B],
                )
                for ql in range(PG):
                    q = dg * PG + ql
                    q0, qsz = chunk[q]
                    if q == q0:
                        ps = pspool.tile([128, qsz, 2 * N], F32, tag="ps")
                        uh = uhpool.tile([128, qsz, N], F32, tag="uh")
                    for c in range(C):
                        nc.tensor.matmul(
                            ps[:, q - q0, :],
                            x_t[:, c, 2 * ql * B : (2 * ql + 2) * B],
                            w_t[:, c, (dg * DG + 2 * ql) * N : (dg * DG + 2 * ql + 2) * N],
                            start=(c == 0),
                            stop=(c == C - 1),
                        )
                    if q == q0 + qsz - 1:
                        # extract diagonal blocks: uh[p<64] = ps[.., 0:N],
                        # uh[p>=64] = ps[.., N:2N]
                        nc.vector.tensor_copy(
                            out=uh[0:64], in_=ps[0:64, :, 0:N]
                        )
                        nc.vector.tensor_copy(
                            out=uh[64:128], in_=ps[64:128, :, N : 2 * N]
                        )
                        routing(uh, vo_full[:, q0 : q0 + qsz], qsz)
            nc.sync.dma_start(out=vout[:, h * P : (h + 1) * P], in_=vo_full)
    nc.finalize()
    return nc


def _prep_core(x, W, k):
    xs = x[2 * k : 2 * k + 2]  # [2, B, S, D]
    # xt[h, p, c, d, b] = x[h, b, c*128+p, d]
    xt = np.ascontiguousarray(
        xs.reshape(H_LOC, B, C, 128, D).transpose(0, 3, 2, 4, 1).astype(IN_NP)
    ).reshape(H_LOC, 128, C, D * B)
    ws = W[2 * k : 2 * k + 2]  # [2, D, N, S]
    # wt[h, p, c, d, n] = W[h, d, n, c*128+p]
    wt = np.ascontiguousarray(
        ws.reshape(H_LOC, D, N, C, 128).transpose(0, 4, 3, 1, 2).astype(IN_NP)
    ).reshape(H_LOC, 128, C, D * N)
    return xt, wt


def _make_ones():
    o = np.zeros((128, 128), dtype=np.float16)
    o[:64, :64] = np.float16(float(N) / B)
    o[64:, 64:] = np.float16(float(N) / B)
    return o


def unshard(vouts):
    """vouts: list of 8 arrays [128, H_LOC*P] -> full [B, D, N, H] output."""
    v_full = np.empty((H, B, D), dtype=np.float32)
    for k in range(NCORES):
        r = vouts[k]  # [128, H_LOC*P]; row = (d%2)*64 + b, col = h*P + d//2
        rr = np.asarray(r).reshape(2, B, H_LOC, P)  # [par, b, h, q]
        for h in range(H_LOC):
            v = np.empty((B, D), dtype=np.float32)
            v[:, 0::2] = rr[0, :, h, :]
            v[:, 1::2] = rr[1, :, h, :]
            v_full[2 * k + h] = v
    out = np.broadcast_to(
        v_full.transpose(1, 2, 0)[:, :, None, :], (B, D, N, H)
    )
    return np.ascontiguousarray(out)


def kernel(x, W, num_routing):
    x = np.asarray(x, dtype=np.float32)
    W = np.asarray(W, dtype=np.float32)
    nr = int(num_routing)
    if nr not in _cache:
        _cache[nr] = _build(nr)
    nc = _cache[nr]

    ones = _make_ones()
    in_maps = []
    for k in range(NCORES):
        xt, wt = _prep_core(x, W, k)
        in_maps.append({"xt": xt, "wt": wt, "ones": ones})

    kernel.last_in_maps = in_maps
    res = run_bass_kernel_spmd(
        nc,
        in_maps,
        core_ids=list(range(NCORES)),
        trace=bool(int(os.environ.get("KERNEL_TRACE", "0"))),
    )
    kernel.last_result = res

    return unshard([res.results[k]["vout"] for k in range(NCORES)])
